# revision 1
# baseline (speedup 1.0000x reference)
"""Trainium2 Bass kernel for: 3x3 conv (reflect pad) + BatchNorm + LeakyReLU + mask.

Input  x:    (1, 64, 512, 512) f32
       W:    (128, 64, 3, 3)   f32
       gamma/beta/mean/var: (128,) f32
       mask: (1, 128, 512, 512) int32 (0/1)
Output (1, 128, 512, 512) f32

Strategy (8 cores, SPMD):
  - Shard H spatially: core c computes output rows [64c, 64c+64).
  - Host reflect-pads x to (64, 514, 514), appends 2 duplicate rows, and ships
    each core TWO bf16 copies of its 67-row slab (second copy shifted down one
    row) stacked into a [128, 67*514] SBUF image. A K=128 matmul against
    stacked weights then computes two conv taps at once:
      partitions   0..63 : channel ci at row y+dy
      partitions 64..127 : channel ci at row y+dy+1
  - 9 taps -> 6 matmuls per output row: 3 "pair" matmuls (dy=0&1, dx=0..2)
    and 3 dy=2 matmuls whose lower 64 weight rows are zero.
  - PSUM accumulates fp32; epilogue = ACT Identity(psum*scale+shift),
    DVE max(z*0.01, z) for LeakyReLU, DVE multiply by uint8 mask.
"""

import numpy as np
import ml_dtypes

import concourse.bacc as bacc
import concourse.bass as bass
import concourse.mybir as mybir
import concourse.tile as tile
from concourse.bass_utils import run_bass_kernel_spmd

bf16 = ml_dtypes.bfloat16

N_CORES = 8
C_IN = 64
C_OUT = 128
H = 512
W_IMG = 512
HS = H // N_CORES            # 64 output rows per core
WP = W_IMG + 2               # 514 padded columns
NROW = HS + 3                # 67 rows per stacked copy
FREE = NROW * WP             # per-partition free elems of the x image
G = 8                        # output rows per pipeline group
LEAK = 0.01
EPS = 1e-5

_CACHE = {}
LAST_RESULTS = None          # BassKernelResults of the last run (for test.py)


def _build_program(hw_lrelu: bool = True) -> bass.Bass:
    """hw_lrelu=True uses the ACT engine's native Lrelu (not implemented in
    CoreSim); False uses an Identity + DVE max(z*a, z) fallback."""
    nc = bacc.Bacc("TRN2", target_bir_lowering=False, debug=False,
                   num_devices=N_CORES)
    f32 = mybir.dt.float32
    bf = mybir.dt.bfloat16
    u8 = mybir.dt.uint8

    xs_d = nc.dram_tensor("xs", [128, FREE], bf, kind="ExternalInput")
    wp_d = nc.dram_tensor("wp", [6, 128, C_OUT], bf, kind="ExternalInput")
    bn_d = nc.dram_tensor("bn", [C_OUT, 2], f32, kind="ExternalInput")
    mk_d = nc.dram_tensor("msk", [C_OUT, HS * W_IMG], u8, kind="ExternalInput")
    out_d = nc.dram_tensor("out", [C_OUT, HS * W_IMG], f32, kind="ExternalOutput")

    with tile.TileContext(nc) as tc:
        with tc.tile_pool(name="const", bufs=1) as cpool, \
             tc.tile_pool(name="xp", bufs=1) as xpool, \
             tc.tile_pool(name="mp", bufs=3) as mpool, \
             tc.tile_pool(name="zp", bufs=4) as zpool, \
             tc.tile_pool(name="op", bufs=3) as opool, \
             tc.tile_pool(name="ps", bufs=8, space="PSUM") as ppool:

            wts = [cpool.tile([128, C_OUT], bf, name=f"w{j}", tag=f"w{j}")
                   for j in range(6)]
            bn = cpool.tile([C_OUT, 2], f32, name="bn_t", tag="bn_t")
            xs = xpool.tile([128, FREE], bf, name="xs_t", tag="xs_t")

            def load_x(r0, r1, eng=None):
                # sync ring (qSPDynamicHW) carries the bulk x stream so
                # stores can never sit ahead of x chunks in its FIFO; the
                # earliest chunks also use the ACT ring (idle until ~25us)
                # for parallel landing
                (eng or nc.sync).dma_start(out=xs[:, r0 * WP:r1 * WP],
                                           in_=xs_d[:, r0 * WP:r1 * WP])

            # weights + bn go on the gpsimd queue so the HWDGE rings carry
            # only the x image early on
            for j in range(6):
                nc.gpsimd.dma_start(out=wts[j][:], in_=wp_d[j, :, :])
            nc.gpsimd.dma_start(out=bn[:], in_=bn_d[:])

            # fine-grained early chunks (land in parallel, PE starts early),
            # coarser later rows
            for i, r0 in enumerate(range(0, 16, 2)):
                load_x(r0, r0 + 2, nc.sync if i % 2 == 0 else nc.scalar)
            for i, r0 in enumerate(range(16, 32, 4)):
                load_x(r0, r0 + 4, nc.sync if i % 2 == 0 else nc.scalar)
            for r0 in range(32, NROW, 8):
                load_x(r0, min(NROW, r0 + 8))

            SG = 4                        # output rows per store tile
            mt = None
            ot = None

            def epilogue(y, pst):
                seg = slice((y % SG) * W_IMG, (y % SG + 1) * W_IMG)
                mseg = slice((y % G) * W_IMG, (y % G + 1) * W_IMG)
                if hw_lrelu:
                    nc.scalar.activation(
                        ot[:, seg], pst[:],
                        mybir.ActivationFunctionType.Lrelu,
                        bias=bn[:, 1:2], scale=bn[:, 0:1], alpha=LEAK)
                else:
                    zt = zpool.tile([C_OUT, W_IMG], f32, name="zt", tag="zt")
                    nc.scalar.activation(
                        zt[:], pst[:],
                        mybir.ActivationFunctionType.Identity,
                        bias=bn[:, 1:2], scale=bn[:, 0:1])
                    nc.vector.scalar_tensor_tensor(
                        ot[:, seg], zt[:], LEAK, zt[:],
                        op0=mybir.AluOpType.mult, op1=mybir.AluOpType.max)
                nc.vector.tensor_tensor(ot[:, seg], ot[:, seg], mt[:, mseg],
                                        op=mybir.AluOpType.mult)
                if y % SG == SG - 1:
                    d0 = (y - SG + 1) * W_IMG
                    # stores ride the ACT HWDGE ring (qActDynamicHW)
                    nc.scalar.dma_start(out=out_d[:, d0:d0 + SG * W_IMG],
                                        in_=ot[:])

            # rows processed in pairs: the three K=64 dy=2 taps of row y run
            # on PE rows 0-63 (copy0) while row y+1's run on rows 64-127
            # (copy1, same flat offset) — disjoint row-groups + separate PSUM
            # banks execute concurrently, so 2 rows cost 9 MM slots, not 12
            for y in range(0, HS, 2):
                if y % G == 0:
                    mt = mpool.tile([C_OUT, G * W_IMG], u8, name="mt", tag="mt")
                    # separate queue (gpsimd/SWDGE): masks must not wait
                    # behind the 8.8 MB x stream on the sync FIFO
                    nc.gpsimd.dma_start(
                        out=mt[:], in_=mk_d[:, y * W_IMG:(y + G) * W_IMG])
                if y % SG == 0:
                    ot = opool.tile([C_OUT, SG * W_IMG], f32, name="ot", tag="ot")
                ps_a = ppool.tile([C_OUT, W_IMG], f32, name="ps_a", tag="pst")
                ps_b = ppool.tile([C_OUT, W_IMG], f32, name="ps_b", tag="pst")
                for yy, ps in ((y, ps_a), (y + 1, ps_b)):
                    for dx in range(3):
                        off = yy * WP + dx
                        nc.tensor.matmul(ps[:], wts[dx][:],
                                         xs[:, off:off + W_IMG],
                                         start=(dx == 0), stop=False)
                for dx in range(3):
                    off = (y + 2) * WP + dx
                    nc.tensor.matmul(ps_a[:], wts[3 + dx][0:64, :],
                                     xs[0:64, off:off + W_IMG],
                                     start=False, stop=(dx == 2))
                    nc.tensor.matmul(ps_b[:], wts[3 + dx][64:128, :],
                                     xs[64:128, off:off + W_IMG],
                                     start=False, stop=(dx == 2))
                epilogue(y, ps_a)
                epilogue(y + 1, ps_b)
    nc.compile()
    return nc


def _get_program(hw_lrelu: bool = True) -> bass.Bass:
    key = ("nc", hw_lrelu)
    if key not in _CACHE:
        _CACHE[key] = _build_program(hw_lrelu)
    return _CACHE[key]


def make_in_maps(x, W, gamma, beta, mean, var, mask):
    """Host-side shard/pack of full inputs into per-core in_maps."""
    x = np.asarray(x, np.float32)
    W = np.asarray(W, np.float32)
    gamma = np.asarray(gamma, np.float32)
    beta = np.asarray(beta, np.float32)
    mean = np.asarray(mean, np.float32)
    var = np.asarray(var, np.float32)
    mask = np.asarray(mask)

    xp = np.pad(x[0], ((0, 0), (1, 1), (1, 1)), mode="reflect")   # [64,514,514]
    xpe = np.concatenate([xp, np.repeat(xp[:, -1:, :], 2, axis=1)], axis=1)
    xpb = xpe.astype(bf16)                                        # [64,516,514]

    wp = np.zeros((6, 128, C_OUT), np.float32)
    for dx in range(3):
        wp[dx, 0:64] = W[:, :, 0, dx].reshape(C_OUT, C_IN).T
        wp[dx, 64:128] = W[:, :, 1, dx].reshape(C_OUT, C_IN).T
        # dy=2 taps duplicated: rows 0-63 serve even rows via copy0,
        # rows 64-127 serve odd rows via copy1 (concurrent row-tiled MMs)
        wp[3 + dx, 0:64] = W[:, :, 2, dx].reshape(C_OUT, C_IN).T
        wp[3 + dx, 64:128] = W[:, :, 2, dx].reshape(C_OUT, C_IN).T
    wp = wp.astype(bf16)

    inv = 1.0 / np.sqrt(var + EPS)
    bn = np.stack([gamma * inv, beta - mean * gamma * inv],
                  axis=1).astype(np.float32)                      # [128,2]

    m8 = mask[0].astype(np.uint8)                                 # [128,512,512]

    in_maps = []
    for c in range(N_CORES):
        S = xpb[:, HS * c:HS * c + HS + 4, :]
        copy0 = np.ascontiguousarray(S[:, 0:NROW, :]).reshape(C_IN, FREE)
        copy1 = np.ascontiguousarray(S[:, 1:NROW + 1, :]).reshape(C_IN, FREE)
        xs_c = np.concatenate([copy0, copy1], axis=0)             # [128, FREE]
        mk_c = np.ascontiguousarray(
            m8[:, HS * c:HS * c + HS, :]).reshape(C_OUT, HS * W_IMG)
        in_maps.append(dict(xs=xs_c, wp=wp, bn=bn, msk=mk_c))
    return in_maps


def kernel(x, W, gamma, beta, mean, var, mask, _trace=False):
    global LAST_RESULTS
    nc = _get_program()
    in_maps = make_in_maps(x, W, gamma, beta, mean, var, mask)
    res = run_bass_kernel_spmd(nc, in_maps, list(range(N_CORES)), trace=_trace)
    LAST_RESULTS = res
    out = np.empty((1, C_OUT, H, W_IMG), np.float32)
    for c in range(N_CORES):
        out[0, :, HS * c:HS * c + HS, :] = \
            np.asarray(res.results[c]["out"]).reshape(C_OUT, HS, W_IMG)
    return out



# revision 2
# speedup vs baseline: 1.1562x; 1.1562x over previous
"""Trainium2 Bass kernel for: 3x3 conv (reflect pad) + BatchNorm + LeakyReLU + mask.

Input  x:    (1, 64, 512, 512) f32
       W:    (128, 64, 3, 3)   f32
       gamma/beta/mean/var: (128,) f32
       mask: (1, 128, 512, 512) int32 (0/1)
Output (1, 128, 512, 512) f32

Strategy (8 cores, SPMD):
  - Shard H spatially: core c computes output rows [64c, 64c+64).
  - Even/odd row interleave, single x copy: host reflect-pads x to
    (64, 514, 514); core c takes its 66-row slab and ships it ONCE as a
    [128, 33*514] bf16 image: partitions 0..63 hold channel ci's EVEN local
    rows (pair index p -> row 2p), partitions 64..127 hold the ODD rows
    (p -> row 2p+1). A K=128 matmul at pair offset p then contracts over two
    adjacent image rows at once (two conv dy taps in one slot).
  - Output row y=2t: pair t covers taps dy=0,1; the lone dy=2 tap (even row
    2t+2) runs as a K=64 matmul on PE rows 0-63. Row y=2t+1: pair t+1 covers
    dy=1,2; lone dy=0 (odd row 2t+1) on PE rows 64-127. The two lone streams
    use disjoint PE row groups -> concurrent, so 2 rows cost 9 matmul slots
    (the algorithmic minimum for 9 taps at K=64 on a K=128 array).
  - 4-row groups, weight-major matmul order: consecutive matmuls reuse the
    same stationary tile so the PE can skip redundant LDWEIGHTS.
  - Epilogue: ACT Lrelu(psum*scale+shift) -> bf16, DVE multiply by uint8
    mask; bf16 stores (harness tolerance 2e-2 >> bf16 rounding).
  - DMA: x+masks on the sync HWDGE ring, weights first + stores on the
    scalar HWDGE ring. No SWDGE.
"""

import numpy as np
import ml_dtypes

import concourse.bacc as bacc
import concourse.bass as bass
import concourse.mybir as mybir
import concourse.tile as tile
from concourse.bass_utils import run_bass_kernel_spmd

bf16 = ml_dtypes.bfloat16

N_CORES = 8
C_IN = 64
C_OUT = 128
H = 512
W_IMG = 512
HS = H // N_CORES            # 64 output rows per core
WP = W_IMG + 2               # 514 padded columns
NPAIR = HS // 2 + 1          # 33 even/odd row pairs per core
FREE = NPAIR * WP            # per-partition free elems of the x image
G = 8                        # output rows per mask tile
SG = 4                       # output rows per store tile / PSUM group
LEAK = 0.01
EPS = 1e-5

_CACHE = {}
LAST_RESULTS = None          # BassKernelResults of the last run (for test.py)


def _build_program(hw_lrelu: bool = True) -> bass.Bass:
    """hw_lrelu=True uses the ACT engine's native Lrelu (not implemented in
    CoreSim); False uses an Identity + DVE max(z*a, z) fallback."""
    nc = bacc.Bacc("TRN2", target_bir_lowering=False, debug=False,
                   num_devices=N_CORES)
    f32 = mybir.dt.float32
    bf = mybir.dt.bfloat16
    u8 = mybir.dt.uint8

    xs_d = nc.dram_tensor("xs", [128, FREE], bf, kind="ExternalInput")
    wp_d = nc.dram_tensor("wp", [128, 9 * C_OUT], bf, kind="ExternalInput")
    bn_d = nc.dram_tensor("bn", [C_OUT, 2], f32, kind="ExternalInput")
    mk_d = nc.dram_tensor("msk", [C_OUT, HS * W_IMG], u8, kind="ExternalInput")
    out_d = nc.dram_tensor("out", [C_OUT, HS * W_IMG], bf, kind="ExternalOutput")

    with tile.TileContext(nc) as tc:
        with tc.tile_pool(name="const", bufs=1) as cpool, \
             tc.tile_pool(name="xp", bufs=1) as xpool, \
             tc.tile_pool(name="mp", bufs=3) as mpool, \
             tc.tile_pool(name="zp", bufs=4) as zpool, \
             tc.tile_pool(name="op", bufs=3) as opool, \
             tc.tile_pool(name="ps", bufs=8, space="PSUM") as ppool:

            wts = cpool.tile([128, 9 * C_OUT], bf, name="wts", tag="wts")
            bn = cpool.tile([C_OUT, 2], f32, name="bn_t", tag="bn_t")
            xs = xpool.tile([128, FREE], bf, name="xs_t", tag="xs_t")

            # weights + bn ride the scalar(ACT) ring ahead of all stores so
            # the PE can start as soon as the first x pairs land
            nc.scalar.dma_start(out=wts[:], in_=wp_d[:])
            nc.scalar.dma_start(out=bn[:], in_=bn_d[:])

            def load_x(p0, p1):
                nc.sync.dma_start(out=xs[:, p0 * WP:p1 * WP],
                                  in_=xs_d[:, p0 * WP:p1 * WP])

            mts = []

            def load_mask(m):
                mt = mpool.tile([C_OUT, G * W_IMG], u8, name="mt", tag="mt")
                nc.sync.dma_start(
                    out=mt[:], in_=mk_d[:, m * G * W_IMG:(m + 1) * G * W_IMG])
                mts.append(mt)

            # sync-ring FIFO: fine-grained early pairs so the PE starts fast,
            # masks interleaved so they arrive well before their group's DVE
            for p in range(4):
                load_x(p, p + 1)
            load_mask(0)
            load_x(4, 8)
            load_x(8, 12)
            load_mask(1)
            load_x(12, 16)
            load_x(16, 20)
            load_mask(2)
            load_x(20, 24)
            load_x(24, 28)
            load_mask(3)
            load_x(28, NPAIR)
            for m in range(4, 8):
                load_mask(m)

            def epilogue(y, pst, ot):
                seg = slice((y % SG) * W_IMG, (y % SG + 1) * W_IMG)
                mt = mts[y // G]
                mseg = slice((y % G) * W_IMG, (y % G + 1) * W_IMG)
                if hw_lrelu:
                    nc.scalar.activation(
                        ot[:, seg], pst[:],
                        mybir.ActivationFunctionType.Lrelu,
                        bias=bn[:, 1:2], scale=bn[:, 0:1], alpha=LEAK)
                else:
                    zt = zpool.tile([C_OUT, W_IMG], f32, name="zt", tag="zt")
                    nc.scalar.activation(
                        zt[:], pst[:],
                        mybir.ActivationFunctionType.Identity,
                        bias=bn[:, 1:2], scale=bn[:, 0:1])
                    nc.vector.scalar_tensor_tensor(
                        ot[:, seg], zt[:], LEAK, zt[:],
                        op0=mybir.AluOpType.mult, op1=mybir.AluOpType.max)
                nc.vector.tensor_tensor(ot[:, seg], ot[:, seg], mt[:, mseg],
                                        op=mybir.AluOpType.mult)

            def w_ap(j):                      # full K=128 stationary tile j
                return wts[:, j * C_OUT:(j + 1) * C_OUT]

            def mm(ps, w, off, start, stop):
                nc.tensor.matmul(ps[:], w, xs[:, off:off + W_IMG],
                                 start=start, stop=stop)

            def mm_h(ps, dx, lo, off, stop):  # K=64 lone-tap matmul
                rows = slice(0, 64) if lo else slice(64, 128)
                nc.tensor.matmul(ps[:], wts[rows, (6 + dx) * C_OUT:(7 + dx) * C_OUT],
                                 xs[rows, off:off + W_IMG],
                                 start=False, stop=stop)

            # 4 output rows per group s: y = 4s..4s+3, pairs t=2s..2s+2.
            #   wA[dx] (block dx):   even rows, pair t(+1): taps dy=0,1
            #   wB[dx] (block 3+dx): odd rows, pair t+1(+2): taps dy=1,2
            #   wC[dx] (block 6+dx): rows 0-63 dy=2 (even lone), 64-127 dy=0
            #     (odd lone); lone streams alternate PE row groups -> overlap.
            # Weight-major order: each stationary tile feeds 2 consecutive
            # matmuls so redundant LDWEIGHTS can be skipped.
            for s in range(HS // SG):
                t = 2 * s
                ot = opool.tile([C_OUT, SG * W_IMG], bf, name="ot", tag="ot")
                ps = [ppool.tile([C_OUT, W_IMG], f32, name=f"ps{i}", tag="pst")
                      for i in range(4)]
                for dx in range(3):
                    mm(ps[0], w_ap(dx), t * WP + dx, dx == 0, False)
                    mm(ps[2], w_ap(dx), (t + 1) * WP + dx, dx == 0, False)
                for dx in range(3):
                    mm(ps[1], w_ap(3 + dx), (t + 1) * WP + dx, dx == 0, False)
                    mm(ps[3], w_ap(3 + dx), (t + 2) * WP + dx, dx == 0, False)
                for dx in range(3):
                    mm_h(ps[0], dx, True, (t + 1) * WP + dx, dx == 2)
                    mm_h(ps[2], dx, True, (t + 2) * WP + dx, dx == 2)
                    mm_h(ps[1], dx, False, t * WP + dx, dx == 2)
                    mm_h(ps[3], dx, False, (t + 1) * WP + dx, dx == 2)
                for i in range(4):
                    epilogue(4 * s + i, ps[i], ot)
                d0 = s * SG * W_IMG
                nc.scalar.dma_start(out=out_d[:, d0:d0 + SG * W_IMG], in_=ot[:])
    nc.compile()
    return nc


def _get_program(hw_lrelu: bool = True) -> bass.Bass:
    key = ("nc", hw_lrelu)
    if key not in _CACHE:
        _CACHE[key] = _build_program(hw_lrelu)
    return _CACHE[key]


def make_in_maps(x, W, gamma, beta, mean, var, mask):
    """Host-side shard/pack of full inputs into per-core in_maps."""
    x = np.asarray(x, np.float32)
    W = np.asarray(W, np.float32)
    gamma = np.asarray(gamma, np.float32)
    beta = np.asarray(beta, np.float32)
    mean = np.asarray(mean, np.float32)
    var = np.asarray(var, np.float32)
    mask = np.asarray(mask)

    xp = np.pad(x[0], ((0, 0), (1, 1), (1, 1)), mode="reflect")   # [64,514,514]
    xpb = xp.astype(bf16)

    # 9 stationary blocks [K=ci, M=co]: see _build_program docstring
    wt = W.transpose(1, 0, 2, 3).astype(np.float32)               # [ci,co,dy,dx]
    wp = np.zeros((128, 9 * C_OUT), np.float32)
    for dx in range(3):
        wp[0:64, dx * C_OUT:(dx + 1) * C_OUT] = wt[:, :, 0, dx]
        wp[64:128, dx * C_OUT:(dx + 1) * C_OUT] = wt[:, :, 1, dx]
        wp[0:64, (3 + dx) * C_OUT:(4 + dx) * C_OUT] = wt[:, :, 1, dx]
        wp[64:128, (3 + dx) * C_OUT:(4 + dx) * C_OUT] = wt[:, :, 2, dx]
        wp[0:64, (6 + dx) * C_OUT:(7 + dx) * C_OUT] = wt[:, :, 2, dx]
        wp[64:128, (6 + dx) * C_OUT:(7 + dx) * C_OUT] = wt[:, :, 0, dx]
    wp = wp.astype(bf16)

    inv = 1.0 / np.sqrt(var + EPS)
    bn = np.stack([gamma * inv, beta - mean * gamma * inv],
                  axis=1).astype(np.float32)                      # [128,2]

    m8 = mask[0].astype(np.uint8)                                 # [128,512,512]

    in_maps = []
    for c in range(N_CORES):
        S = xpb[:, HS * c:HS * c + HS + 2, :]                     # 66 rows
        even = np.ascontiguousarray(S[:, 0::2, :]).reshape(C_IN, FREE)
        odd = np.ascontiguousarray(S[:, 1::2, :]).reshape(C_IN, FREE)
        xs_c = np.concatenate([even, odd], axis=0)                # [128, FREE]
        mk_c = np.ascontiguousarray(
            m8[:, HS * c:HS * c + HS, :]).reshape(C_OUT, HS * W_IMG)
        in_maps.append(dict(xs=xs_c, wp=wp, bn=bn, msk=mk_c))
    return in_maps


def kernel(x, W, gamma, beta, mean, var, mask, _trace=False):
    global LAST_RESULTS
    nc = _get_program()
    in_maps = make_in_maps(x, W, gamma, beta, mean, var, mask)
    res = run_bass_kernel_spmd(nc, in_maps, list(range(N_CORES)), trace=_trace)
    LAST_RESULTS = res
    out = np.empty((1, C_OUT, H, W_IMG), np.float32)
    for c in range(N_CORES):
        out[0, :, HS * c:HS * c + HS, :] = \
            np.asarray(res.results[c]["out"]).astype(np.float32) \
              .reshape(C_OUT, HS, W_IMG)
    return out


# revision 4
# speedup vs baseline: 1.1787x; 1.0195x over previous
"""Trainium2 Bass kernel for: 3x3 conv (reflect pad) + BatchNorm + LeakyReLU + mask.

Input  x:    (1, 64, 512, 512) f32
       W:    (128, 64, 3, 3)   f32
       gamma/beta/mean/var: (128,) f32
       mask: (1, 128, 512, 512) int32 (0/1)
Output (1, 128, 512, 512) f32

Strategy (8 cores, SPMD):
  - Shard H spatially: core c computes output rows [64c, 64c+64).
  - Even/odd row interleave, single x copy: host reflect-pads x to
    (64, 514, 514); core c takes its 66-row slab and ships it ONCE as a
    [128, 33*514] bf16 image: partitions 0..63 hold channel ci's EVEN local
    rows (pair index p -> row 2p), partitions 64..127 hold the ODD rows
    (p -> row 2p+1). A K=128 matmul at pair offset p then contracts over two
    adjacent image rows at once (two conv dy taps in one slot).
  - Output row y=2t: pair t covers taps dy=0,1; the lone dy=2 tap (even row
    2t+2) runs as a K=64 matmul on PE rows 0-63. Row y=2t+1: pair t+1 covers
    dy=1,2; lone dy=0 (odd row 2t+1) on PE rows 64-127. The two lone streams
    use disjoint PE row groups -> concurrent, so 2 rows cost 9 matmul slots
    (the algorithmic minimum for 9 taps at K=64 on a K=128 array).
  - 4-row groups, weight-major matmul order: consecutive matmuls reuse the
    same stationary tile so the PE can skip redundant LDWEIGHTS.
  - Epilogue: ACT Lrelu(psum*scale+shift) -> bf16, DVE multiply by uint8
    mask; bf16 stores (harness tolerance 2e-2 >> bf16 rounding).
  - DMA: x+masks on the sync HWDGE ring, weights first + stores on the
    scalar HWDGE ring. No SWDGE.
"""

import numpy as np
import ml_dtypes

import concourse.bacc as bacc
import concourse.bass as bass
import concourse.mybir as mybir
import concourse.tile as tile
from concourse.bass_utils import run_bass_kernel_spmd

bf16 = ml_dtypes.bfloat16

N_CORES = 8
C_IN = 64
C_OUT = 128
H = 512
W_IMG = 512
HS = H // N_CORES            # 64 output rows per core
WP = W_IMG + 2               # 514 padded columns
NPAIR = HS // 2 + 1          # 33 even/odd row pairs per core
FREE = NPAIR * WP            # per-partition free elems of the x image
G = 8                        # output rows per mask tile
SG = 4                       # output rows per store tile / PSUM group
LEAK = 0.01
EPS = 1e-5

_CACHE = {}
LAST_RESULTS = None          # BassKernelResults of the last run (for test.py)


def _build_program(hw_lrelu: bool = True) -> bass.Bass:
    """hw_lrelu=True uses the ACT engine's native Lrelu (not implemented in
    CoreSim); False uses an Identity + DVE max(z*a, z) fallback."""
    nc = bacc.Bacc("TRN2", target_bir_lowering=False, debug=False,
                   num_devices=N_CORES)
    f32 = mybir.dt.float32
    bf = mybir.dt.bfloat16
    u8 = mybir.dt.uint8

    xs_d = nc.dram_tensor("xs", [128, FREE], bf, kind="ExternalInput")
    wp_d = nc.dram_tensor("wp", [128, 9 * C_OUT], bf, kind="ExternalInput")
    bn_d = nc.dram_tensor("bn", [C_OUT, 2], f32, kind="ExternalInput")
    mk_d = nc.dram_tensor("msk", [C_OUT, HS * W_IMG], u8, kind="ExternalInput")
    out_d = nc.dram_tensor("out", [C_OUT, HS * W_IMG], bf, kind="ExternalOutput")

    with tile.TileContext(nc) as tc:
        with tc.tile_pool(name="const", bufs=1) as cpool, \
             tc.tile_pool(name="xp", bufs=1) as xpool, \
             tc.tile_pool(name="mp", bufs=3) as mpool, \
             tc.tile_pool(name="zp", bufs=4) as zpool, \
             tc.tile_pool(name="op", bufs=4) as opool, \
             tc.tile_pool(name="ps", bufs=8, space="PSUM") as ppool:

            wts = cpool.tile([128, 9 * C_OUT], bf, name="wts", tag="wts")
            bn = cpool.tile([C_OUT, 2], f32, name="bn_t", tag="bn_t")
            xs = xpool.tile([128, FREE], bf, name="xs_t", tag="xs_t")

            def load_w(j0, j1, eng):
                eng.dma_start(out=wts[:, j0 * C_OUT:j1 * C_OUT],
                              in_=wp_d[:, j0 * C_OUT:j1 * C_OUT])

            def load_x(p0, p1):
                nc.sync.dma_start(out=xs[:, p0 * WP:p1 * WP],
                                  in_=xs_d[:, p0 * WP:p1 * WP])

            mts = []

            def load_mask(m):
                mt = mpool.tile([C_OUT, G * W_IMG], u8, name="mt", tag="mt")
                nc.sync.dma_start(
                    out=mt[:], in_=mk_d[:, m * G * W_IMG:(m + 1) * G * W_IMG])
                mts.append(mt)

            # sync-ring FIFO: the first stationary block + fine-grained early
            # pairs land first so the PE starts fast; masks interleaved so
            # they arrive well before their group's DVE. The remaining weight
            # blocks + bn ride the scalar(ACT) ring in parallel (the scalar
            # ring carries nothing else until the first ACTIVATE).
            load_w(0, 1, nc.sync)
            load_w(1, 9, nc.scalar)
            nc.scalar.dma_start(out=bn[:], in_=bn_d[:])
            for p in range(4):
                load_x(p, p + 1)
            load_mask(0)
            load_x(4, 8)
            load_x(8, 12)
            load_mask(1)
            load_x(12, 16)
            load_x(16, 20)
            load_mask(2)
            load_x(20, 24)
            load_x(24, 28)
            load_mask(3)
            load_x(28, NPAIR)
            for m in range(4, 8):
                load_mask(m)

            def epilogue(y, pst, ot):
                seg = slice((y % SG) * W_IMG, (y % SG + 1) * W_IMG)
                mt = mts[y // G]
                mseg = slice((y % G) * W_IMG, (y % G + 1) * W_IMG)
                if hw_lrelu:
                    nc.scalar.activation(
                        ot[:, seg], pst[:],
                        mybir.ActivationFunctionType.Lrelu,
                        bias=bn[:, 1:2], scale=bn[:, 0:1], alpha=LEAK)
                else:
                    zt = zpool.tile([C_OUT, W_IMG], f32, name="zt", tag="zt")
                    nc.scalar.activation(
                        zt[:], pst[:],
                        mybir.ActivationFunctionType.Identity,
                        bias=bn[:, 1:2], scale=bn[:, 0:1])
                    nc.vector.scalar_tensor_tensor(
                        ot[:, seg], zt[:], LEAK, zt[:],
                        op0=mybir.AluOpType.mult, op1=mybir.AluOpType.max)
                nc.vector.tensor_tensor(ot[:, seg], ot[:, seg], mt[:, mseg],
                                        op=mybir.AluOpType.mult)

            def w_ap(j):                      # full K=128 stationary tile j
                return wts[:, j * C_OUT:(j + 1) * C_OUT]

            def mm(ps, w, off, start, stop):
                nc.tensor.matmul(ps[:], w, xs[:, off:off + W_IMG],
                                 start=start, stop=stop)

            def mm_h(ps, dx, lo, off, stop):  # K=64 lone-tap matmul
                rows = slice(0, 64) if lo else slice(64, 128)
                nc.tensor.matmul(ps[:], wts[rows, (6 + dx) * C_OUT:(7 + dx) * C_OUT],
                                 xs[rows, off:off + W_IMG],
                                 start=False, stop=stop)

            # 4 output rows per group s: y = 4s..4s+3, pairs t=2s..2s+2.
            #   wA[dx] (block dx):   even rows, pair t(+1): taps dy=0,1
            #   wB[dx] (block 3+dx): odd rows, pair t+1(+2): taps dy=1,2
            #   wC[dx] (block 6+dx): rows 0-63 dy=2 (even lone), 64-127 dy=0
            #     (odd lone); lone streams alternate PE row groups -> overlap.
            # Weight-major order: each stationary tile feeds 2 consecutive
            # matmuls so redundant LDWEIGHTS can be skipped.
            # Stores ride the sync ring: the scalar sequencer is near its
            # limit on ACTIVATEs alone and DMA_DIRECT2D issue costs ~590ns.
            for s in range(HS // SG - 1):
                t = 2 * s
                ot = opool.tile([C_OUT, SG * W_IMG], bf, name="ot", tag="ot")
                ps = [ppool.tile([C_OUT, W_IMG], f32, name=f"ps{i}", tag="pst")
                      for i in range(4)]
                for dx in range(3):
                    mm(ps[0], w_ap(dx), t * WP + dx, dx == 0, False)
                    mm(ps[2], w_ap(dx), (t + 1) * WP + dx, dx == 0, False)
                for dx in range(3):
                    mm(ps[1], w_ap(3 + dx), (t + 1) * WP + dx, dx == 0, False)
                    mm(ps[3], w_ap(3 + dx), (t + 2) * WP + dx, dx == 0, False)
                for dx in range(3):
                    mm_h(ps[0], dx, True, (t + 1) * WP + dx, dx == 2)
                    mm_h(ps[2], dx, True, (t + 2) * WP + dx, dx == 2)
                    mm_h(ps[1], dx, False, t * WP + dx, dx == 2)
                    mm_h(ps[3], dx, False, (t + 1) * WP + dx, dx == 2)
                for i in range(4):
                    epilogue(4 * s + i, ps[i], ot)
                d0 = s * SG * W_IMG
                nc.sync.dma_start(out=out_d[:, d0:d0 + SG * W_IMG], in_=ot[:])

            # last 4 rows: complete each row's PSUM as early as possible and
            # store per-row, so the post-matmul tail is one ACT + one DVE +
            # one 128KB store instead of 4 serial ACTs + a 512KB store
            for y in range(HS - SG, HS):
                pst = ppool.tile([C_OUT, W_IMG], f32, name="ps_l", tag="pst")
                if y % 2 == 0:
                    t = y // 2
                    for dx in range(3):
                        mm(pst, w_ap(dx), t * WP + dx, dx == 0, False)
                    for dx in range(3):
                        mm_h(pst, dx, True, (t + 1) * WP + dx, dx == 2)
                else:
                    t = (y - 1) // 2
                    for dx in range(3):
                        mm(pst, w_ap(3 + dx), (t + 1) * WP + dx, dx == 0, False)
                    for dx in range(3):
                        mm_h(pst, dx, False, t * WP + dx, dx == 2)
                ot = opool.tile([C_OUT, W_IMG], bf, name="otl", tag="otl")
                mt = mts[y // G]
                mseg = slice((y % G) * W_IMG, (y % G + 1) * W_IMG)
                if hw_lrelu:
                    nc.scalar.activation(
                        ot[:], pst[:], mybir.ActivationFunctionType.Lrelu,
                        bias=bn[:, 1:2], scale=bn[:, 0:1], alpha=LEAK)
                else:
                    zt = zpool.tile([C_OUT, W_IMG], f32, name="zt", tag="zt")
                    nc.scalar.activation(
                        zt[:], pst[:], mybir.ActivationFunctionType.Identity,
                        bias=bn[:, 1:2], scale=bn[:, 0:1])
                    nc.vector.scalar_tensor_tensor(
                        ot[:], zt[:], LEAK, zt[:],
                        op0=mybir.AluOpType.mult, op1=mybir.AluOpType.max)
                nc.vector.tensor_tensor(ot[:], ot[:], mt[:, mseg],
                                        op=mybir.AluOpType.mult)
                nc.sync.dma_start(out=out_d[:, y * W_IMG:(y + 1) * W_IMG],
                                  in_=ot[:])
    nc.compile()
    return nc


def _get_program(hw_lrelu: bool = True) -> bass.Bass:
    key = ("nc", hw_lrelu)
    if key not in _CACHE:
        _CACHE[key] = _build_program(hw_lrelu)
    return _CACHE[key]


def make_in_maps(x, W, gamma, beta, mean, var, mask):
    """Host-side shard/pack of full inputs into per-core in_maps."""
    x = np.asarray(x, np.float32)
    W = np.asarray(W, np.float32)
    gamma = np.asarray(gamma, np.float32)
    beta = np.asarray(beta, np.float32)
    mean = np.asarray(mean, np.float32)
    var = np.asarray(var, np.float32)
    mask = np.asarray(mask)

    xp = np.pad(x[0], ((0, 0), (1, 1), (1, 1)), mode="reflect")   # [64,514,514]
    xpb = xp.astype(bf16)

    # 9 stationary blocks [K=ci, M=co]: see _build_program docstring
    wt = W.transpose(1, 0, 2, 3).astype(np.float32)               # [ci,co,dy,dx]
    wp = np.zeros((128, 9 * C_OUT), np.float32)
    for dx in range(3):
        wp[0:64, dx * C_OUT:(dx + 1) * C_OUT] = wt[:, :, 0, dx]
        wp[64:128, dx * C_OUT:(dx + 1) * C_OUT] = wt[:, :, 1, dx]
        wp[0:64, (3 + dx) * C_OUT:(4 + dx) * C_OUT] = wt[:, :, 1, dx]
        wp[64:128, (3 + dx) * C_OUT:(4 + dx) * C_OUT] = wt[:, :, 2, dx]
        wp[0:64, (6 + dx) * C_OUT:(7 + dx) * C_OUT] = wt[:, :, 2, dx]
        wp[64:128, (6 + dx) * C_OUT:(7 + dx) * C_OUT] = wt[:, :, 0, dx]
    wp = wp.astype(bf16)

    inv = 1.0 / np.sqrt(var + EPS)
    bn = np.stack([gamma * inv, beta - mean * gamma * inv],
                  axis=1).astype(np.float32)                      # [128,2]

    m8 = mask[0].astype(np.uint8)                                 # [128,512,512]

    in_maps = []
    for c in range(N_CORES):
        S = xpb[:, HS * c:HS * c + HS + 2, :]                     # 66 rows
        even = np.ascontiguousarray(S[:, 0::2, :]).reshape(C_IN, FREE)
        odd = np.ascontiguousarray(S[:, 1::2, :]).reshape(C_IN, FREE)
        xs_c = np.concatenate([even, odd], axis=0)                # [128, FREE]
        mk_c = np.ascontiguousarray(
            m8[:, HS * c:HS * c + HS, :]).reshape(C_OUT, HS * W_IMG)
        in_maps.append(dict(xs=xs_c, wp=wp, bn=bn, msk=mk_c))
    return in_maps


def kernel(x, W, gamma, beta, mean, var, mask, _trace=False):
    global LAST_RESULTS
    nc = _get_program()
    in_maps = make_in_maps(x, W, gamma, beta, mean, var, mask)
    res = run_bass_kernel_spmd(nc, in_maps, list(range(N_CORES)), trace=_trace)
    LAST_RESULTS = res
    out = np.empty((1, C_OUT, H, W_IMG), np.float32)
    for c in range(N_CORES):
        out[0, :, HS * c:HS * c + HS, :] = \
            np.asarray(res.results[c]["out"]).astype(np.float32) \
              .reshape(C_OUT, HS, W_IMG)
    return out


# revision 8
# speedup vs baseline: 1.1920x; 1.0113x over previous
"""Trainium2 Bass kernel for: 3x3 conv (reflect pad) + BatchNorm + LeakyReLU + mask.

Input  x:    (1, 64, 512, 512) f32
       W:    (128, 64, 3, 3)   f32
       gamma/beta/mean/var: (128,) f32
       mask: (1, 128, 512, 512) int32 (0/1)
Output (1, 128, 512, 512) f32

Strategy (8 cores, SPMD):
  - Shard H spatially: core c computes output rows [64c, 64c+64).
  - Even/odd row interleave, single x copy: host reflect-pads x to
    (64, 514, 514); core c takes its 66-row slab and ships it ONCE as a
    [128, 33*514] bf16 image: partitions 0..63 hold channel ci's EVEN local
    rows (pair index p -> row 2p), partitions 64..127 hold the ODD rows
    (p -> row 2p+1). A K=128 matmul at pair offset p then contracts over two
    adjacent image rows at once (two conv dy taps in one slot).
  - Output row y=2t: pair t covers taps dy=0,1; the lone dy=2 tap (even row
    2t+2) runs as a K=64 matmul on PE rows 0-63. Row y=2t+1: pair t+1 covers
    dy=1,2; lone dy=0 (odd row 2t+1) on PE rows 64-127. The two lone streams
    use disjoint PE row groups -> concurrent, so 2 rows cost 9 matmul slots
    (the algorithmic minimum for 9 taps at K=64 on a K=128 array).
  - 4-row groups, weight-major matmul order: consecutive matmuls reuse the
    same stationary tile so the PE can skip redundant LDWEIGHTS.
  - Epilogue: ACT Lrelu(psum*scale+shift) -> bf16, DVE multiply by uint8
    mask; bf16 stores (harness tolerance 2e-2 >> bf16 rounding).
  - DMA: x+masks on the sync HWDGE ring, weights first + stores on the
    scalar HWDGE ring. No SWDGE.
"""

import numpy as np
import ml_dtypes

import concourse.bacc as bacc
import concourse.bass as bass
import concourse.mybir as mybir
import concourse.tile as tile
from concourse.bass_utils import run_bass_kernel_spmd

bf16 = ml_dtypes.bfloat16

N_CORES = 8
C_IN = 64
C_OUT = 128
H = 512
W_IMG = 512
HS = H // N_CORES            # 64 output rows per core
WP = W_IMG + 2               # 514 padded columns
NPAIR = HS // 2 + 1          # 33 even/odd row pairs per core
FREE = NPAIR * WP            # per-partition free elems of the x image
G = 8                        # output rows per mask tile
SG = 4                       # output rows per store tile / PSUM group
LEAK = 0.01
EPS = 1e-5

_CACHE = {}
LAST_RESULTS = None          # BassKernelResults of the last run (for test.py)


def _build_program(hw_lrelu: bool = True) -> bass.Bass:
    """hw_lrelu=True uses the ACT engine's native Lrelu (not implemented in
    CoreSim); False uses an Identity + DVE max(z*a, z) fallback."""
    nc = bacc.Bacc("TRN2", target_bir_lowering=False, debug=False,
                   num_devices=N_CORES)
    f32 = mybir.dt.float32
    bf = mybir.dt.bfloat16
    u8 = mybir.dt.uint8

    xs_d = nc.dram_tensor("xs", [128, FREE], bf, kind="ExternalInput")
    wp_d = nc.dram_tensor("wp", [128, 9 * C_OUT], bf, kind="ExternalInput")
    bn_d = nc.dram_tensor("bn", [C_OUT, 2], f32, kind="ExternalInput")
    mk_d = nc.dram_tensor("msk", [C_OUT, HS * W_IMG], u8, kind="ExternalInput")
    out_d = nc.dram_tensor("out", [C_OUT, HS * W_IMG], bf, kind="ExternalOutput")

    with tile.TileContext(nc) as tc:
        with tc.tile_pool(name="const", bufs=1) as cpool, \
             tc.tile_pool(name="xp", bufs=1) as xpool, \
             tc.tile_pool(name="mp", bufs=3) as mpool, \
             tc.tile_pool(name="zp", bufs=4) as zpool, \
             tc.tile_pool(name="op", bufs=4) as opool, \
             tc.tile_pool(name="ps", bufs=8, space="PSUM") as ppool:

            # block 0 is its own tile: the first matmul must depend only on
            # its 32KB DMA, not on the bulk weight transfer
            w0 = cpool.tile([128, C_OUT], bf, name="w0", tag="w0")
            wts = cpool.tile([128, 8 * C_OUT], bf, name="wts", tag="wts")
            bn = cpool.tile([C_OUT, 2], f32, name="bn_t", tag="bn_t")
            xs = xpool.tile([128, FREE], bf, name="xs_t", tag="xs_t")

            def load_x(p0, p1):
                nc.sync.dma_start(out=xs[:, p0 * WP:p1 * WP],
                                  in_=xs_d[:, p0 * WP:p1 * WP])

            mts = []

            def load_mask(m):
                mt = mpool.tile([C_OUT, G * W_IMG], u8, name="mt", tag="mt")
                nc.sync.dma_start(
                    out=mt[:], in_=mk_d[:, m * G * W_IMG:(m + 1) * G * W_IMG])
                mts.append(mt)

            # sync-ring FIFO: the first stationary block + fine-grained early
            # pairs land first so the PE starts fast; masks interleaved so
            # they arrive well before their group's DVE. The remaining weight
            # blocks + bn ride the scalar(ACT) ring in parallel (the scalar
            # ring carries nothing else until the first ACTIVATE).
            nc.sync.dma_start(out=w0[:], in_=wp_d[:, 0:C_OUT])
            nc.scalar.dma_start(out=wts[:], in_=wp_d[:, C_OUT:9 * C_OUT])
            nc.scalar.dma_start(out=bn[:], in_=bn_d[:])
            for p in range(4):
                load_x(p, p + 1)
            load_mask(0)
            load_x(4, 8)
            load_x(8, 12)
            load_mask(1)
            load_x(12, 16)
            load_x(16, 20)
            load_mask(2)
            load_x(20, 24)
            load_x(24, 28)
            load_mask(3)
            load_x(28, NPAIR)
            for m in range(4, 8):
                load_mask(m)

            def epilogue(y, pst, ot):
                seg = slice((y % SG) * W_IMG, (y % SG + 1) * W_IMG)
                mt = mts[y // G]
                mseg = slice((y % G) * W_IMG, (y % G + 1) * W_IMG)
                if hw_lrelu:
                    nc.scalar.activation(
                        ot[:, seg], pst[:],
                        mybir.ActivationFunctionType.Lrelu,
                        bias=bn[:, 1:2], scale=bn[:, 0:1], alpha=LEAK)
                else:
                    zt = zpool.tile([C_OUT, W_IMG], f32, name="zt", tag="zt")
                    nc.scalar.activation(
                        zt[:], pst[:],
                        mybir.ActivationFunctionType.Identity,
                        bias=bn[:, 1:2], scale=bn[:, 0:1])
                    nc.vector.scalar_tensor_tensor(
                        ot[:, seg], zt[:], LEAK, zt[:],
                        op0=mybir.AluOpType.mult, op1=mybir.AluOpType.max)
                nc.vector.tensor_tensor(ot[:, seg], ot[:, seg], mt[:, mseg],
                                        op=mybir.AluOpType.mult)

            def w_ap(j):                      # full K=128 stationary tile j
                if j == 0:
                    return w0[:]
                return wts[:, (j - 1) * C_OUT:j * C_OUT]

            def mm(ps, w, off, start, stop):
                nc.tensor.matmul(ps[:], w, xs[:, off:off + W_IMG],
                                 start=start, stop=stop)

            def mm_h(ps, dx, lo, off, stop):  # K=64 lone-tap matmul
                rows = slice(0, 64) if lo else slice(64, 128)
                nc.tensor.matmul(ps[:], wts[rows, (5 + dx) * C_OUT:(6 + dx) * C_OUT],
                                 xs[rows, off:off + W_IMG],
                                 start=False, stop=stop)

            # 4 output rows per group s: y = 4s..4s+3, pairs t=2s..2s+2.
            #   wA[dx] (block dx):   even rows, pair t(+1): taps dy=0,1
            #   wB[dx] (block 3+dx): odd rows, pair t+1(+2): taps dy=1,2
            #   wC[dx] (block 6+dx): rows 0-63 dy=2 (even lone), 64-127 dy=0
            #     (odd lone); lone streams alternate PE row groups -> overlap.
            # Weight-major order: each stationary tile feeds 2 consecutive
            # matmuls so redundant LDWEIGHTS can be skipped.
            # Stores ride the sync ring: the scalar sequencer is near its
            # limit on ACTIVATEs alone and DMA_DIRECT2D issue costs ~590ns.
            for s in range(HS // SG - 1):
                t = 2 * s
                ot = opool.tile([C_OUT, SG * W_IMG], bf, name="ot", tag="ot")
                ps = [ppool.tile([C_OUT, W_IMG], f32, name=f"ps{i}", tag="pst")
                      for i in range(4)]
                for dx in range(3):
                    mm(ps[0], w_ap(dx), t * WP + dx, dx == 0, False)
                    mm(ps[2], w_ap(dx), (t + 1) * WP + dx, dx == 0, False)
                for dx in range(3):
                    mm(ps[1], w_ap(3 + dx), (t + 1) * WP + dx, dx == 0, False)
                    mm(ps[3], w_ap(3 + dx), (t + 2) * WP + dx, dx == 0, False)
                for dx in range(3):
                    mm_h(ps[0], dx, True, (t + 1) * WP + dx, dx == 2)
                    mm_h(ps[2], dx, True, (t + 2) * WP + dx, dx == 2)
                    mm_h(ps[1], dx, False, t * WP + dx, dx == 2)
                    mm_h(ps[3], dx, False, (t + 1) * WP + dx, dx == 2)
                for i in range(4):
                    epilogue(4 * s + i, ps[i], ot)
                d0 = s * SG * W_IMG
                nc.sync.dma_start(out=out_d[:, d0:d0 + SG * W_IMG], in_=ot[:])

            # last 4 rows: two 2-row paired subgroups (keeps the lone-tap PE
            # row-group overlap) with per-row stores on the otherwise-idle
            # scalar ring, so the post-matmul tail is short and the final
            # store doesn't queue behind earlier bulk stores on the sync ring
            for y0 in range(HS - SG, HS, 2):
                t = y0 // 2
                ps_a = ppool.tile([C_OUT, W_IMG], f32, name="ps_la", tag="pst")
                ps_b = ppool.tile([C_OUT, W_IMG], f32, name="ps_lb", tag="pst")
                for dx in range(3):
                    mm(ps_a, w_ap(dx), t * WP + dx, dx == 0, False)
                for dx in range(3):
                    mm(ps_b, w_ap(3 + dx), (t + 1) * WP + dx, dx == 0, False)
                for dx in range(3):
                    mm_h(ps_a, dx, True, (t + 1) * WP + dx, dx == 2)
                    mm_h(ps_b, dx, False, t * WP + dx, dx == 2)
                for y, pst in ((y0, ps_a), (y0 + 1, ps_b)):
                    ot = opool.tile([C_OUT, W_IMG], bf, name="otl", tag="otl")
                    mt = mts[y // G]
                    mseg = slice((y % G) * W_IMG, (y % G + 1) * W_IMG)
                    if hw_lrelu:
                        nc.scalar.activation(
                            ot[:], pst[:], mybir.ActivationFunctionType.Lrelu,
                            bias=bn[:, 1:2], scale=bn[:, 0:1], alpha=LEAK)
                    else:
                        zt = zpool.tile([C_OUT, W_IMG], f32, name="zt", tag="zt")
                        nc.scalar.activation(
                            zt[:], pst[:], mybir.ActivationFunctionType.Identity,
                            bias=bn[:, 1:2], scale=bn[:, 0:1])
                        nc.vector.scalar_tensor_tensor(
                            ot[:], zt[:], LEAK, zt[:],
                            op0=mybir.AluOpType.mult, op1=mybir.AluOpType.max)
                    nc.vector.tensor_tensor(ot[:], ot[:], mt[:, mseg],
                                            op=mybir.AluOpType.mult)
                    nc.scalar.dma_start(out=out_d[:, y * W_IMG:(y + 1) * W_IMG],
                                        in_=ot[:])
    nc.compile()
    return nc


def _get_program(hw_lrelu: bool = True) -> bass.Bass:
    key = ("nc", hw_lrelu)
    if key not in _CACHE:
        _CACHE[key] = _build_program(hw_lrelu)
    return _CACHE[key]


def make_in_maps(x, W, gamma, beta, mean, var, mask):
    """Host-side shard/pack of full inputs into per-core in_maps."""
    x = np.asarray(x, np.float32)
    W = np.asarray(W, np.float32)
    gamma = np.asarray(gamma, np.float32)
    beta = np.asarray(beta, np.float32)
    mean = np.asarray(mean, np.float32)
    var = np.asarray(var, np.float32)
    mask = np.asarray(mask)

    xp = np.pad(x[0], ((0, 0), (1, 1), (1, 1)), mode="reflect")   # [64,514,514]
    xpb = xp.astype(bf16)

    # 9 stationary blocks [K=ci, M=co]: see _build_program docstring
    wt = W.transpose(1, 0, 2, 3).astype(np.float32)               # [ci,co,dy,dx]
    wp = np.zeros((128, 9 * C_OUT), np.float32)
    for dx in range(3):
        wp[0:64, dx * C_OUT:(dx + 1) * C_OUT] = wt[:, :, 0, dx]
        wp[64:128, dx * C_OUT:(dx + 1) * C_OUT] = wt[:, :, 1, dx]
        wp[0:64, (3 + dx) * C_OUT:(4 + dx) * C_OUT] = wt[:, :, 1, dx]
        wp[64:128, (3 + dx) * C_OUT:(4 + dx) * C_OUT] = wt[:, :, 2, dx]
        wp[0:64, (6 + dx) * C_OUT:(7 + dx) * C_OUT] = wt[:, :, 2, dx]
        wp[64:128, (6 + dx) * C_OUT:(7 + dx) * C_OUT] = wt[:, :, 0, dx]
    wp = wp.astype(bf16)

    inv = 1.0 / np.sqrt(var + EPS)
    bn = np.stack([gamma * inv, beta - mean * gamma * inv],
                  axis=1).astype(np.float32)                      # [128,2]

    m8 = mask[0].astype(np.uint8)                                 # [128,512,512]

    in_maps = []
    for c in range(N_CORES):
        S = xpb[:, HS * c:HS * c + HS + 2, :]                     # 66 rows
        even = np.ascontiguousarray(S[:, 0::2, :]).reshape(C_IN, FREE)
        odd = np.ascontiguousarray(S[:, 1::2, :]).reshape(C_IN, FREE)
        xs_c = np.concatenate([even, odd], axis=0)                # [128, FREE]
        mk_c = np.ascontiguousarray(
            m8[:, HS * c:HS * c + HS, :]).reshape(C_OUT, HS * W_IMG)
        in_maps.append(dict(xs=xs_c, wp=wp, bn=bn, msk=mk_c))
    return in_maps


def kernel(x, W, gamma, beta, mean, var, mask, _trace=False):
    global LAST_RESULTS
    nc = _get_program()
    in_maps = make_in_maps(x, W, gamma, beta, mean, var, mask)
    res = run_bass_kernel_spmd(nc, in_maps, list(range(N_CORES)), trace=_trace)
    LAST_RESULTS = res
    out = np.empty((1, C_OUT, H, W_IMG), np.float32)
    for c in range(N_CORES):
        out[0, :, HS * c:HS * c + HS, :] = \
            np.asarray(res.results[c]["out"]).astype(np.float32) \
              .reshape(C_OUT, HS, W_IMG)
    return out
